# revision 35
# baseline (speedup 1.0000x reference)
"""DogeCDMoE Trainium2 kernel: product-key MoE routing + dense MLP.

Strategy (8 NeuronCores, data-parallel over the 4096 tokens, 512 each):
  - Host: compose `keys` into W_q so routing scores come from ONE bf16 PE
    matmul; pre-transpose weights; expert tables and hs cast to fp8 e4m3
    with power-of-2 scales so the two expert-side matmuls (all-expert
    logits, expert combine) run in fp8 DoubleRow mode (two 128-deep
    K-chunks contracted per pass). The dense MLP stays bf16 (fp8 there
    fails the 2e-2 error gate; verified in numpy emulation).
  - Device per core:
      sim   = hsT.T @ WK           (bf16 PE, [tok, 512] routing scores)
      The routing chain (top-8 per axis via DVE max8/max_index, cartesian
      top-8, expert-id rebuild, softmax) runs batched across heads and is
      emitted AFTER all sim matmuls so the PE stream (sim -> logits ->
      up-proj -> down-proj -> combine) never waits on DVE.
      All 4 heads' probs (pre-scaled by the fp8 descale 2^-9) merge into
      ONE prob map per chunk: duplicate experts carry the summed prob at
      every duplicate slot, late duplicates get negative scatter indices
      (ignored); GPSIMD local_scatter builds pmap [tok, 4096].
      Lg    = logits for ALL experts (fp8 DoubleRow PE, [tok, 4096]),
              single pass over expert columns with all 4 token chunks
              inside so de streams once; 4 lg buffers let routing and the
              w-chains run strictly after B, off the PE critical path.
      sc    = silu(Lg * pmap)       (ONE multiply + ONE silu per chunk)
      S.T via DMA-transpose bounce, then cast to fp8 (x64).
      out.T = W_down.T(x2^16)-matmul(silu(up)) + ue(x2^10)-matmul(S.T x2^6),
              accumulated in the same PSUM banks, output scaled by 2^-16.
  - DMA queues: every big weight stream is issued on the sync queue in
    exact consumption order (hsTb, de, wup, wdown, ue) so FIFO head-of-line
    blocking never delays a phase; residents + transposes go on the scalar
    queue, output on the gpsimd queue; streams are batched (wup 4-wide,
    wd/ue 2-wide) since each dma_start carries a fixed ~0.6us issue cost.
"""

import numpy as np
import ml_dtypes

B, T, H = 2, 2048, 1024
I = 4096
HEADS = 4
RET = 128
E = 4096           # NUM_EXPERTS
NK = 64            # NUM_KEYS
K = 8
NCORES = 8
NT = (B * T) // NCORES   # 512 tokens per core
P = 128
TCH = NT // P            # 4 token chunks
HK = H // P              # 8 contraction chunks over H
ICH = I // P             # 32 chunks over intermediate / expert dim
DCH = H // P             # 8 output d-chunks
QE = 1024                # local_scatter quarter size over expert dim

# fp8 power-of-2 scales
HS_SC = 8.0        # hidden_states
DE_SC = 64.0       # down_embed (logits descale 2^-9 folded into probs)
S_SC = 64.0        # silu weights S
UE_SC = 1024.0     # up_embed
OUT_DESC = 1.0 / (S_SC * UE_SC)   # 2^-16; W_down pre-scaled by 2^16

_CACHE = {}


def _build_program(repeat=1, loop_n=None):
    """Build the program. `repeat` unrolls the body; `loop_n` instead wraps it
    in a tc.For_i hardware loop (used by bench_hw.py for repeat-delta timing
    with a compact program)."""
    from contextlib import ExitStack
    import concourse.tile as tile
    from concourse import bacc, mybir

    nc = bacc.Bacc("TRN2", target_bir_lowering=False, debug=False)
    f32 = mybir.dt.float32
    bf16 = mybir.dt.bfloat16
    fp8 = mybir.dt.float8e4
    i32 = mybir.dt.int32
    i16 = mybir.dt.int16
    u32 = mybir.dt.uint32
    AF = mybir.ActivationFunctionType
    OP = mybir.AluOpType
    AX = mybir.AxisListType
    DR = mybir.MatmulPerfMode.DoubleRow

    # ---- I/O ----
    hsT_b = nc.dram_tensor("hsT_b", [H, NT], bf16, kind="ExternalInput")
    hsT_q = nc.dram_tensor("hsT_q", [H, NT], fp8, kind="ExternalInput")
    wk_d = nc.dram_tensor("wk", [H, 512], bf16, kind="ExternalInput")
    wupT_d = nc.dram_tensor("wupT", [H, I], bf16, kind="ExternalInput")
    wdownT_d = nc.dram_tensor("wdownT", [I, H], bf16, kind="ExternalInput")
    deq_d = nc.dram_tensor("deq", [H, E], fp8, kind="ExternalInput")
    ueq_d = nc.dram_tensor("ueq", [E, H], fp8, kind="ExternalInput")
    outT_d = nc.dram_tensor("outT", [H, NT], f32, kind="ExternalOutput")

    wk_r = wk_d[:].rearrange("(o p) e -> p o e", p=P)
    hsTb_r = hsT_b[:].rearrange("(o p) n -> p o n", p=P)
    hsTq_r = hsT_q[:].rearrange("(o t p) n -> p o t n", p=P, t=2)
    deq_r = deq_d[:].rearrange("(o t p) e -> p o t e", p=P, t=2)
    wupT_r = wupT_d[:].rearrange("(o p) i -> p o i", p=P)
    wdownT_r = wdownT_d[:].rearrange("(o p) d -> p o d", p=P)
    ueq_r = ueq_d[:].rearrange("(o t p) d -> p o t d", p=P, t=2)

    with tile.TileContext(nc) as tc, ExitStack() as ctx:
        res = ctx.enter_context(tc.tile_pool(name="res", bufs=1))
        streams = ctx.enter_context(tc.tile_pool(name="streams", bufs=3))
        wupstr = ctx.enter_context(tc.tile_pool(name="wupstr", bufs=2))
        wstream = ctx.enter_context(tc.tile_pool(name="wstream", bufs=2))
        rpool = ctx.enter_context(tc.tile_pool(name="rpool", bufs=2))
        eqpool = ctx.enter_context(tc.tile_pool(name="eqpool", bufs=1))
        lgpool = ctx.enter_context(tc.tile_pool(name="lgpool", bufs=4))
        pmpool = ctx.enter_context(tc.tile_pool(name="pmpool", bufs=1))
        scpool = ctx.enter_context(tc.tile_pool(name="scpool", bufs=1))
        outp = ctx.enter_context(tc.tile_pool(name="outp", bufs=2))
        psum = ctx.enter_context(tc.tile_pool(name="psum", bufs=8, space="PSUM"))

        # ---------- residents ----------
        iota8 = res.tile([P, 8], i32)
        nc.gpsimd.iota(iota8[:], pattern=[[1, 8]], base=0, channel_multiplier=0)
        iota32 = res.tile([P, 32], i32)
        nc.gpsimd.iota(iota32[:], pattern=[[1, 32]], base=0, channel_multiplier=0)
        qoff = res.tile([P, 4], i32)
        nc.gpsimd.iota(qoff[:], pattern=[[QE, 4]], base=QE, channel_multiplier=0)
        # lower-triangular mask lt[p,i,j] = (i > j)
        ltf = res.tile([P, 32, 32], bf16)
        nc.vector.tensor_tensor(ltf[:], iota32[:, :, None].to_broadcast([P, 32, 32]),
                                iota32[:, None, :].to_broadcast([P, 32, 32]),
                                OP.is_gt)

        # split the startup loads so phase A's first matmul starts early
        hsTb_sb = res.tile([P, HK, NT], bf16)
        nc.sync.dma_start(hsTb_sb[:, :, :NT // 2], hsTb_r[:, :, :NT // 2])
        nc.sync.dma_start(hsTb_sb[:, :, NT // 2:], hsTb_r[:, :, NT // 2:])
        wk_sb = res.tile([P, HK, 512], bf16)
        nc.scalar.dma_start(wk_sb[:, :HK // 2, :], wk_r[:, :HK // 2, :])
        nc.scalar.dma_start(wk_sb[:, HK // 2:, :], wk_r[:, HK // 2:, :])
        hsq_sb = res.tile([P, HK // 2, 2, NT], fp8)
        nc.scalar.dma_start(hsq_sb[:], hsTq_r[:])

        sim_all = res.tile([P, TCH, 512], f32)     # routing scores
        y1T = res.tile([P, ICH, NT], bf16)         # silu(up-proj), I on partitions
        sTq = res.tile([P, ICH, NT], fp8)          # S.T fp8 (x S_SC)

        # routing results that survive to the scatter phase
        pv_all = res.tile([P, TCH, 32], bf16)      # scatter values (summed probs)
        idx_all = res.tile([P, TCH, 4, 32], i16)   # per-quarter scatter indices

        def _routing(c):
            """Batched-over-heads routing for token chunk c (DVE + a little
            ACT); writes pv_all[:, c] and idx_all[:, c]."""
            sx = rpool.tile([P, HEADS, 8], f32, tag="sx")
            sy = rpool.tile([P, HEADS, 8], f32, tag="sy")
            ix = rpool.tile([P, HEADS, 8], u32, tag="ix")
            iy = rpool.tile([P, HEADS, 8], u32, tag="iy")
            for h in range(HEADS):
                simx = sim_all[:, c, h * NK:(h + 1) * NK]
                simy = sim_all[:, c, 256 + h * NK:256 + (h + 1) * NK]
                nc.vector.max(sx[:, h, :], simx)
                nc.vector.max_index(ix[:, h, :], sx[:, h, :], simx)
                nc.vector.max(sy[:, h, :], simy)
                nc.vector.max_index(iy[:, h, :], sy[:, h, :], simy)

            cc = eqpool.tile([P, HEADS, 8, 8], f32, tag="cc")
            nc.vector.tensor_tensor(cc[:], sx[:, :, :, None].to_broadcast([P, HEADS, 8, 8]),
                                    sy[:, :, None, :].to_broadcast([P, HEADS, 8, 8]),
                                    OP.add)
            s8 = rpool.tile([P, HEADS, 8], f32, tag="s8")
            pk = rpool.tile([P, HEADS, 8], u32, tag="pk")
            for h in range(HEADS):
                cflat = cc[:, h, :, :].rearrange("p a b -> p (a b)")
                nc.vector.max(s8[:, h, :], cflat)
                nc.vector.max_index(pk[:, h, :], s8[:, h, :], cflat)

            # softmax over the 8 selected scores; fold in the 2^-9 descale
            d8 = rpool.tile([P, HEADS, 8], f32, tag="d8")
            nc.vector.tensor_tensor(d8[:], s8[:],
                                    s8[:, :, 0:1].to_broadcast([P, HEADS, 8]),
                                    OP.subtract)
            ex8 = rpool.tile([P, HEADS, 8], f32, tag="ex8")
            nc.scalar.activation(ex8[:], d8[:], AF.Exp)
            z = rpool.tile([P, HEADS], f32, tag="z")
            nc.vector.tensor_reduce(z[:], ex8[:], axis=AX.X, op=OP.add)
            rz = rpool.tile([P, HEADS], f32, tag="rz")
            nc.vector.reciprocal(rz[:], z[:])
            p8v = rpool.tile([P, HEADS, 8], f32, tag="p8v")
            nc.vector.scalar_tensor_tensor(p8v[:], ex8[:], 2.0 ** -9,
                                           rz[:, :, None].to_broadcast([P, HEADS, 8]),
                                           op0=OP.mult, op1=OP.mult)

            # expert ids: e8 = ix[pk>>3]*64 + iy[pk&7], batched across heads
            pkhu = rpool.tile([P, HEADS, 8], u32, tag="pkhu")
            pklu = rpool.tile([P, HEADS, 8], u32, tag="pklu")
            nc.vector.tensor_scalar(pkhu[:], pk[:], 3, None, op0=OP.logical_shift_right)
            nc.vector.tensor_scalar(pklu[:], pk[:], 7, None, op0=OP.bitwise_and)
            pkh = rpool.tile([P, HEADS, 8], i32, tag="pkh")
            pkl = rpool.tile([P, HEADS, 8], i32, tag="pkl")
            nc.vector.tensor_copy(pkh[:], pkhu[:])
            nc.vector.tensor_copy(pkl[:], pklu[:])
            ixi = rpool.tile([P, HEADS, 8], i32, tag="ixi")
            iyi = rpool.tile([P, HEADS, 8], i32, tag="iyi")
            nc.vector.tensor_copy(ixi[:], ix[:])
            nc.vector.tensor_copy(iyi[:], iy[:])

            ohx = eqpool.tile([P, HEADS, 8, 8], i32, tag="ohx")
            ohy = eqpool.tile([P, HEADS, 8, 8], i32, tag="ohy")
            nc.vector.tensor_tensor(ohx[:], pkh[:, :, :, None].to_broadcast([P, HEADS, 8, 8]),
                                    iota8[:, None, None, :].to_broadcast([P, HEADS, 8, 8]),
                                    OP.is_equal)
            nc.vector.tensor_tensor(ohy[:], pkl[:, :, :, None].to_broadcast([P, HEADS, 8, 8]),
                                    iota8[:, None, None, :].to_broadcast([P, HEADS, 8, 8]),
                                    OP.is_equal)
            mx = eqpool.tile([P, HEADS, 8, 8], i32, tag="mx")
            my = eqpool.tile([P, HEADS, 8, 8], i32, tag="my")
            nc.vector.tensor_tensor(mx[:], ohx[:],
                                    ixi[:, :, None, :].to_broadcast([P, HEADS, 8, 8]),
                                    OP.mult)
            nc.vector.tensor_tensor(my[:], ohy[:],
                                    iyi[:, :, None, :].to_broadcast([P, HEADS, 8, 8]),
                                    OP.mult)
            ixs = rpool.tile([P, HEADS, 8], i32, tag="ixs")
            iys = rpool.tile([P, HEADS, 8], i32, tag="iys")
            e8i = rpool.tile([P, HEADS, 8], i32, tag="e8i")
            with nc.allow_low_precision(reason="int32 onehot-select, exact"):
                nc.vector.tensor_reduce(ixs[:], mx[:], axis=AX.X, op=OP.add)
                nc.vector.tensor_reduce(iys[:], my[:], axis=AX.X, op=OP.add)
            nc.vector.scalar_tensor_tensor(e8i[:], ixs[:], NK, iys[:],
                                           op0=OP.mult, op1=OP.add)

            # merge heads: summed probs at duplicate experts, late dups killed
            e32 = e8i[:].rearrange("p h k -> p (h k)")
            p32 = p8v[:].rearrange("p h k -> p (h k)")
            eqf = eqpool.tile([P, 32, 32], f32, tag="eqf")
            nc.vector.tensor_tensor(eqf[:], e32[:, :, None].to_broadcast([P, 32, 32]),
                                    e32[:, None, :].to_broadcast([P, 32, 32]),
                                    OP.is_equal)
            pmm = eqpool.tile([P, 32, 32], f32, tag="pmm")
            nc.vector.tensor_tensor(pmm[:], eqf[:],
                                    p32[:, None, :].to_broadcast([P, 32, 32]), OP.mult)
            ptot = rpool.tile([P, 32], f32, tag="ptot")
            nc.vector.tensor_reduce(ptot[:], pmm[:], axis=AX.X, op=OP.add)
            nc.vector.tensor_copy(pv_all[:, c, :], ptot[:])

            eql = eqpool.tile([P, 32, 32], f32, tag="pmm")
            nc.vector.tensor_tensor(eql[:], eqf[:], ltf[:], OP.mult)
            dup = rpool.tile([P, 32], f32, tag="dup")
            nc.vector.tensor_reduce(dup[:], eql[:], axis=AX.X, op=OP.max)
            dupi = rpool.tile([P, 32], i32, tag="dupi")
            nc.vector.tensor_copy(dupi[:], dup[:])
            dupneg = rpool.tile([P, 32], i32, tag="dupneg")
            nc.vector.tensor_scalar(dupneg[:], dupi[:], -4096, None, op0=OP.mult)

            # quarter-local scatter indices, wrong-quarter/late-dup -> negative
            t1 = rpool.tile([P, 4, 32], i32, tag="t1")
            nc.vector.tensor_tensor(t1[:], e32[:, None, :].to_broadcast([P, 4, 32]),
                                    qoff[:, :, None].to_broadcast([P, 4, 32]),
                                    OP.subtract)
            t2 = rpool.tile([P, 4, 32], i32, tag="t2")
            nc.vector.tensor_scalar(t2[:], t1[:], 4095, None, op0=OP.bitwise_and)
            t3 = rpool.tile([P, 4, 32], i32, tag="t3")
            nc.vector.tensor_scalar(t3[:], t2[:], 3072, None, op0=OP.subtract)
            with nc.allow_low_precision(reason="int32 index math, exact"):
                nc.vector.tensor_tensor(idx_all[:, c, :, :], t3[:],
                                        dupneg[:, None, :].to_broadcast([P, 4, 32]),
                                        OP.add)

        def _emit_body():
            # ---------- phase A: sim matmuls (PE) + immediate DVE copies ----
            for c in range(TCH):
                ps = psum.tile([P, 512], f32, tag="ps")
                for kk in range(HK):
                    nc.tensor.matmul(ps[:], hsTb_sb[:, kk, c * P:(c + 1) * P],
                                     wk_sb[:, kk, :],
                                     start=(kk == 0), stop=(kk == HK - 1))
                nc.vector.tensor_copy(sim_all[:, c, :], ps[:])
            # ---------- phase B: all-expert logits (fp8 DoubleRow) ----------
            # single pass over expert columns with all 4 token chunks inside:
            # de is streamed once (4MB); the 4 lg buffers hold every chunk's
            # logits so routing + w-chains run strictly after B on DVE/ACT/
            # GPSIMD while the PE moves on to phase C.
            lg_t = [lgpool.tile([P, E], bf16, tag="lg", name=f"lg{c}")
                    for c in range(TCH)]
            for ec in range(E // 512):
                de_t = streams.tile([P, HK // 2, 2, 512], fp8, tag="de_t")
                nc.sync.dma_start(de_t[:], deq_r[:, :, :, ec * 512:(ec + 1) * 512])
                for c in range(TCH):
                    ps = psum.tile([P, 512], f32, tag="ps")
                    for j in range(HK // 2):
                        nc.tensor.matmul(ps[:], hsq_sb[:, j, :, c * P:(c + 1) * P],
                                         de_t[:, j, :, :], start=(j == 0),
                                         stop=(j == HK // 2 - 1), perf_mode=DR)
                    nc.vector.tensor_copy(lg_t[c][:, ec * 512:(ec + 1) * 512], ps[:])

            # routing (DVE) + w-chains (GPSIMD scatter, DVE mult/cast, ACT silu,
            # xbar transpose) — all off the PE's critical path
            for c in range(TCH):
                _routing(c)
            for c in range(TCH):
                pmap = pmpool.tile([P, E], bf16, tag="pmap")
                for qq in range(4):
                    nc.gpsimd.local_scatter(
                        pmap[:, qq * QE:(qq + 1) * QE], pv_all[:, c, :],
                        idx_all[:, c, qq, :], channels=P, num_elems=QE,
                        num_idxs=32)
                tt = scpool.tile([P, E], bf16, tag="tt")
                nc.vector.tensor_tensor(tt[:], lg_t[c][:], pmap[:], OP.mult)
                nc.scalar.activation(tt[:], tt[:], AF.Silu)
                sTb = scpool.tile([P, ICH, P], bf16, tag="sTb")
                # quarter-granularity transposes so stream DMAs interleave
                # on the shared DMA engines instead of stalling ~3.6us
                for tq in range(4):
                    nc.scalar.dma_start_transpose(
                        sTb[:, tq * (ICH // 4):(tq + 1) * (ICH // 4), :],
                        tt[:, tq * (E // 4):(tq + 1) * (E // 4)])
                nc.vector.tensor_scalar(sTq[:, :, c * P:(c + 1) * P], sTb[:],
                                        S_SC, None, op0=OP.mult)

            # ---------- phase C: dense up-proj + silu ----------
            for ic8 in range(ICH // 4):
                wup_t = wupstr.tile([P, HK, 4 * P], bf16, tag="wup_t")
                nc.sync.dma_start(wup_t[:], wupT_r[:, :, ic8 * 4 * P:(ic8 + 1) * 4 * P])
                for j in range(4):
                    ic = ic8 * 4 + j
                    ps = psum.tile([P, 512], f32, tag="ps")
                    for kk in range(HK):
                        nc.tensor.matmul(ps[:], wup_t[:, kk, j * P:(j + 1) * P],
                                         hsTb_sb[:, kk, :],
                                         start=(kk == 0), stop=(kk == HK - 1))
                    nc.scalar.activation(y1T[:, ic, :], ps[:], AF.Silu)

            # ---------- phase D: down-proj + expert combine, fused in PSUM ----------
            # All 8 output d-chunks accumulate concurrently (8 PSUM banks) so each
            # weight row-block is loaded once, in one large DMA.
            ps_d = [psum.tile([P, 512], f32, tag="ps", name=f"ps_d{dc}")
                    for dc in range(DCH)]
            ue0 = None
            for ic2 in range(ICH // 2):
                wd_t = wstream.tile([P, 2, H], bf16, tag="wd_t")
                nc.sync.dma_start(wd_t[:], wdownT_r[:, 2 * ic2:2 * ic2 + 2, :])
                for i2 in range(2):
                    ic = 2 * ic2 + i2
                    for dc in range(DCH):
                        nc.tensor.matmul(ps_d[dc][:], wd_t[:, i2, dc * P:(dc + 1) * P],
                                         y1T[:, ic, :], start=(ic == 0), stop=False)
                if ic2 == ICH // 2 - 4:
                    # hoist the first ue load past the tail of the wd stream so
                    # the combine phase starts without waiting on the sync FIFO
                    ue0 = wstream.tile([P, 2, 2, H], fp8, tag="ue_t", name="ue0")
                    nc.sync.dma_start(ue0[:], ueq_r[:, 0:2, :, :])
            for j2 in range(E // 512):
                if j2 == 0:
                    ue_t = ue0
                else:
                    ue_t = wstream.tile([P, 2, 2, H], fp8, tag="ue_t")
                    nc.sync.dma_start(ue_t[:], ueq_r[:, 2 * j2:2 * j2 + 2, :, :])
                for i2 in range(2):
                    j = 2 * j2 + i2
                    for dc in range(DCH):
                        nc.tensor.matmul(ps_d[dc][:], ue_t[:, i2, :, dc * P:(dc + 1) * P],
                                         sTq[:, 2 * j:2 * j + 2, :], start=False,
                                         stop=(j == E // 256 - 1), perf_mode=DR)
            for dc in range(DCH):
                ot = outp.tile([P, 512], f32, tag="ot")
                nc.scalar.activation(ot[:], ps_d[dc][:], AF.Copy, scale=OUT_DESC)
                nc.gpsimd.dma_start(outT_d[dc * P:(dc + 1) * P, :], ot[:])

        if loop_n is not None:
            with tc.For_i(0, loop_n, 1):
                _emit_body()
        else:
            for _rep in range(repeat):
                _emit_body()

    nc.compile()
    return nc


def _host_prep(hidden_states, W_up, W_down, W_q, keys, down_embed, up_embed):
    bf = ml_dtypes.bfloat16
    f8 = ml_dtypes.float8_e4m3
    hs = np.asarray(hidden_states, dtype=np.float32).reshape(B * T, H)
    W_up = np.asarray(W_up, dtype=np.float32)
    W_down = np.asarray(W_down, dtype=np.float32)
    W_q = np.asarray(W_q, dtype=np.float32)
    keys = np.asarray(keys, dtype=np.float32)
    down_embed = np.asarray(down_embed, dtype=np.float32)
    up_embed = np.asarray(up_embed, dtype=np.float32)

    # compose product-key similarity: WK[(p2,h,k), d] = sum_r Wq[(p2,h,r), d]*keys[h,k,p2,r]
    Wq3 = W_q.reshape(2, HEADS, NK, H).astype(np.float64)
    WK = np.einsum("phrd,hkpr->phkd", Wq3, keys.astype(np.float64))
    WK_T = np.ascontiguousarray(WK.reshape(512, H).T).astype(np.float32)  # [H, 512]

    shared = {
        "wk": WK_T.astype(bf),
        "wupT": np.ascontiguousarray(W_up.T).astype(bf),                   # [H, I]
        "wdownT": np.ascontiguousarray(W_down.T * (S_SC * UE_SC)).astype(bf),
        "deq": np.ascontiguousarray(down_embed.T * DE_SC).astype(f8),      # [H, E]
        "ueq": np.ascontiguousarray(up_embed * UE_SC).astype(f8),          # [E, H]
    }
    in_maps = []
    for i in range(NCORES):
        shard = hs[i * NT:(i + 1) * NT]                              # [NT, H]
        hsT = np.ascontiguousarray(shard.T)                          # [H, NT]
        m = dict(shared)
        m["hsT_b"] = hsT.astype(bf)
        m["hsT_q"] = (hsT * HS_SC).astype(f8)
        in_maps.append(m)
    return in_maps


def kernel(hidden_states, W_up, W_down, W_q, keys, down_embed, up_embed,
           trace=False):
    from concourse.bass_utils import run_bass_kernel_spmd

    if "nc" not in _CACHE:
        _CACHE["nc"] = _build_program()
    nc = _CACHE["nc"]

    in_maps = _host_prep(hidden_states, W_up, W_down, W_q, keys,
                         down_embed, up_embed)
    res = run_bass_kernel_spmd(nc, in_maps, list(range(NCORES)), trace=trace)
    out = np.empty((B * T, H), np.float32)
    for i, r in enumerate(res.results):
        out[i * NT:(i + 1) * NT] = r["outT"].T
    if trace:
        kernel.last_results = res
    return out.reshape(B, T, H)


# revision 36
# speedup vs baseline: 1.0439x; 1.0439x over previous
"""DogeCDMoE Trainium2 kernel: product-key MoE routing + dense MLP.

Strategy (8 NeuronCores, data-parallel over the 4096 tokens, 512 each):
  - Host: compose `keys` into W_q so routing scores come from ONE bf16 PE
    matmul; pre-transpose weights; expert tables and hs cast to fp8 e4m3
    with power-of-2 scales so the two expert-side matmuls (all-expert
    logits, expert combine) run in fp8 DoubleRow mode (two 128-deep
    K-chunks contracted per pass). The dense MLP stays bf16 (fp8 there
    fails the 2e-2 error gate; verified in numpy emulation).
  - Device per core:
      sim   = hsT.T @ WK           (bf16 PE, [tok, 512] routing scores)
      The routing chain (top-8 per axis via DVE max8/max_index, cartesian
      top-8, expert-id rebuild, softmax) runs batched across heads and is
      emitted AFTER all sim matmuls so the PE stream (sim -> logits ->
      up-proj -> down-proj -> combine) never waits on DVE.
      All 4 heads' probs (pre-scaled by the fp8 descale 2^-9) merge into
      ONE prob map per chunk: duplicate experts carry the summed prob at
      every duplicate slot, late duplicates get negative scatter indices
      (ignored); GPSIMD local_scatter builds pmap [tok, 4096].
      Lg    = logits for ALL experts (fp8 DoubleRow PE, [tok, 4096]),
              single pass over expert columns with all 4 token chunks
              inside so de streams once; 4 lg buffers let routing and the
              w-chains run strictly after B, off the PE critical path.
      sc    = silu(Lg * pmap)       (ONE multiply + ONE silu per chunk)
      S.T via DMA-transpose bounce, then cast to fp8 (x64).
      out.T = W_down.T(x2^16)-matmul(silu(up)) + ue(x2^10)-matmul(S.T x2^6),
              accumulated in the same PSUM banks, output scaled by 2^-16.
  - DMA queues: every big weight stream is issued on the sync queue in
    exact consumption order (hsTb, de, wup, wdown, ue) so FIFO head-of-line
    blocking never delays a phase; residents + transposes go on the scalar
    queue, output on the gpsimd queue; streams are batched (wup 4-wide,
    wd/ue 2-wide) since each dma_start carries a fixed ~0.6us issue cost.
"""

import numpy as np
import ml_dtypes

B, T, H = 2, 2048, 1024
I = 4096
HEADS = 4
RET = 128
E = 4096           # NUM_EXPERTS
NK = 64            # NUM_KEYS
K = 8
NCORES = 8
NT = (B * T) // NCORES   # 512 tokens per core
P = 128
TCH = NT // P            # 4 token chunks
HK = H // P              # 8 contraction chunks over H
ICH = I // P             # 32 chunks over intermediate / expert dim
DCH = H // P             # 8 output d-chunks
QE = 1024                # local_scatter quarter size over expert dim

# fp8 power-of-2 scales
HS_SC = 8.0        # hidden_states
DE_SC = 64.0       # down_embed (logits descale 2^-9 folded into probs)
S_SC = 64.0        # silu weights S
UE_SC = 1024.0     # up_embed
OUT_DESC = 1.0 / (S_SC * UE_SC)   # 2^-16; W_down pre-scaled by 2^16

_CACHE = {}


def _build_program(repeat=1, loop_n=None):
    """Build the program. `repeat` unrolls the body; `loop_n` instead wraps it
    in a tc.For_i hardware loop (used by bench_hw.py for repeat-delta timing
    with a compact program)."""
    from contextlib import ExitStack
    import concourse.tile as tile
    from concourse import bacc, mybir

    nc = bacc.Bacc("TRN2", target_bir_lowering=False, debug=False)
    f32 = mybir.dt.float32
    bf16 = mybir.dt.bfloat16
    fp8 = mybir.dt.float8e4
    i32 = mybir.dt.int32
    i16 = mybir.dt.int16
    u32 = mybir.dt.uint32
    AF = mybir.ActivationFunctionType
    OP = mybir.AluOpType
    AX = mybir.AxisListType
    DR = mybir.MatmulPerfMode.DoubleRow

    # ---- I/O ----
    hsT_b = nc.dram_tensor("hsT_b", [H, NT], bf16, kind="ExternalInput")
    hsT_q = nc.dram_tensor("hsT_q", [H, NT], fp8, kind="ExternalInput")
    wk_d = nc.dram_tensor("wk", [H, 512], bf16, kind="ExternalInput")
    wupT_d = nc.dram_tensor("wupT", [H, I], bf16, kind="ExternalInput")
    wdownT_d = nc.dram_tensor("wdownT", [I, H], bf16, kind="ExternalInput")
    deq_d = nc.dram_tensor("deq", [H, E], fp8, kind="ExternalInput")
    ueq_d = nc.dram_tensor("ueq", [E, H], fp8, kind="ExternalInput")
    outT_d = nc.dram_tensor("outT", [H, NT], f32, kind="ExternalOutput")

    wk_r = wk_d[:].rearrange("(o p) e -> p o e", p=P)
    hsTb_r = hsT_b[:].rearrange("(o p) n -> p o n", p=P)
    hsTq_r = hsT_q[:].rearrange("(o t p) n -> p o t n", p=P, t=2)
    deq_r = deq_d[:].rearrange("(o t p) e -> p o t e", p=P, t=2)
    wupT_r = wupT_d[:].rearrange("(o p) i -> p o i", p=P)
    wdownT_r = wdownT_d[:].rearrange("(o p) d -> p o d", p=P)
    ueq_r = ueq_d[:].rearrange("(o t p) d -> p o t d", p=P, t=2)

    with tile.TileContext(nc) as tc, ExitStack() as ctx:
        res = ctx.enter_context(tc.tile_pool(name="res", bufs=1))
        streams = ctx.enter_context(tc.tile_pool(name="streams", bufs=3))
        wupstr = ctx.enter_context(tc.tile_pool(name="wupstr", bufs=2))
        wstream = ctx.enter_context(tc.tile_pool(name="wstream", bufs=2))
        rpool = ctx.enter_context(tc.tile_pool(name="rpool", bufs=2))
        eqpool = ctx.enter_context(tc.tile_pool(name="eqpool", bufs=1))
        lgpool = ctx.enter_context(tc.tile_pool(name="lgpool", bufs=4))
        pmpool = ctx.enter_context(tc.tile_pool(name="pmpool", bufs=1))
        scpool = ctx.enter_context(tc.tile_pool(name="scpool", bufs=1))
        outp = ctx.enter_context(tc.tile_pool(name="outp", bufs=2))
        psum = ctx.enter_context(tc.tile_pool(name="psum", bufs=8, space="PSUM"))

        # ---------- residents ----------
        iota8 = res.tile([P, 8], i32)
        nc.gpsimd.iota(iota8[:], pattern=[[1, 8]], base=0, channel_multiplier=0)
        iota32 = res.tile([P, 32], i32)
        nc.gpsimd.iota(iota32[:], pattern=[[1, 32]], base=0, channel_multiplier=0)
        qoff = res.tile([P, 4], i32)
        nc.gpsimd.iota(qoff[:], pattern=[[QE, 4]], base=QE, channel_multiplier=0)
        # lower-triangular mask lt[p,i,j] = (i > j)
        ltf = res.tile([P, 32, 32], bf16)
        nc.vector.tensor_tensor(ltf[:], iota32[:, :, None].to_broadcast([P, 32, 32]),
                                iota32[:, None, :].to_broadcast([P, 32, 32]),
                                OP.is_gt)

        # split the startup loads so phase A's first matmul starts early
        hsTb_sb = res.tile([P, HK, NT], bf16)
        nc.sync.dma_start(hsTb_sb[:, :, :NT // 2], hsTb_r[:, :, :NT // 2])
        nc.sync.dma_start(hsTb_sb[:, :, NT // 2:], hsTb_r[:, :, NT // 2:])
        wk_sb = res.tile([P, HK, 512], bf16)
        nc.scalar.dma_start(wk_sb[:, :HK // 2, :], wk_r[:, :HK // 2, :])
        nc.scalar.dma_start(wk_sb[:, HK // 2:, :], wk_r[:, HK // 2:, :])
        hsq_sb = res.tile([P, HK // 2, 2, NT], fp8)
        nc.scalar.dma_start(hsq_sb[:], hsTq_r[:])

        sim_all = res.tile([P, TCH, 512], f32)     # routing scores
        y1T = res.tile([P, ICH, NT], bf16)         # silu(up-proj), I on partitions
        sTq = res.tile([P, ICH, NT], fp8)          # S.T fp8 (x S_SC)

        # routing results that survive to the scatter phase
        pv_all = res.tile([P, TCH, 32], bf16)      # scatter values (summed probs)
        idx_all = res.tile([P, TCH, 4, 32], i16)   # per-quarter scatter indices

        def _routing(c):
            """Batched-over-heads routing for token chunk c (DVE + a little
            ACT); writes pv_all[:, c] and idx_all[:, c]."""
            sx = rpool.tile([P, HEADS, 8], f32, tag="sx")
            sy = rpool.tile([P, HEADS, 8], f32, tag="sy")
            ix = rpool.tile([P, HEADS, 8], u32, tag="ix")
            iy = rpool.tile([P, HEADS, 8], u32, tag="iy")
            for h in range(HEADS):
                simx = sim_all[:, c, h * NK:(h + 1) * NK]
                simy = sim_all[:, c, 256 + h * NK:256 + (h + 1) * NK]
                nc.vector.max(sx[:, h, :], simx)
                nc.vector.max_index(ix[:, h, :], sx[:, h, :], simx)
                nc.vector.max(sy[:, h, :], simy)
                nc.vector.max_index(iy[:, h, :], sy[:, h, :], simy)

            cc = eqpool.tile([P, HEADS, 8, 8], f32, tag="cc")
            nc.vector.tensor_tensor(cc[:], sx[:, :, :, None].to_broadcast([P, HEADS, 8, 8]),
                                    sy[:, :, None, :].to_broadcast([P, HEADS, 8, 8]),
                                    OP.add)
            s8 = rpool.tile([P, HEADS, 8], f32, tag="s8")
            pk = rpool.tile([P, HEADS, 8], u32, tag="pk")
            for h in range(HEADS):
                cflat = cc[:, h, :, :].rearrange("p a b -> p (a b)")
                nc.vector.max(s8[:, h, :], cflat)
                nc.vector.max_index(pk[:, h, :], s8[:, h, :], cflat)

            # softmax over the 8 selected scores; fold in the 2^-9 descale
            d8 = rpool.tile([P, HEADS, 8], f32, tag="d8")
            nc.vector.tensor_tensor(d8[:], s8[:],
                                    s8[:, :, 0:1].to_broadcast([P, HEADS, 8]),
                                    OP.subtract)
            ex8 = rpool.tile([P, HEADS, 8], f32, tag="ex8")
            nc.scalar.activation(ex8[:], d8[:], AF.Exp)
            z = rpool.tile([P, HEADS], f32, tag="z")
            nc.vector.tensor_reduce(z[:], ex8[:], axis=AX.X, op=OP.add)
            rz = rpool.tile([P, HEADS], f32, tag="rz")
            nc.vector.reciprocal(rz[:], z[:])
            p8v = rpool.tile([P, HEADS, 8], f32, tag="p8v")
            nc.vector.scalar_tensor_tensor(p8v[:], ex8[:], 2.0 ** -9,
                                           rz[:, :, None].to_broadcast([P, HEADS, 8]),
                                           op0=OP.mult, op1=OP.mult)

            # expert ids: e8 = ix[pk>>3]*64 + iy[pk&7], batched across heads
            pkhu = rpool.tile([P, HEADS, 8], u32, tag="pkhu")
            pklu = rpool.tile([P, HEADS, 8], u32, tag="pklu")
            nc.vector.tensor_scalar(pkhu[:], pk[:], 3, None, op0=OP.logical_shift_right)
            nc.vector.tensor_scalar(pklu[:], pk[:], 7, None, op0=OP.bitwise_and)
            pkh = rpool.tile([P, HEADS, 8], i32, tag="pkh")
            pkl = rpool.tile([P, HEADS, 8], i32, tag="pkl")
            nc.vector.tensor_copy(pkh[:], pkhu[:])
            nc.vector.tensor_copy(pkl[:], pklu[:])
            ixi = rpool.tile([P, HEADS, 8], i32, tag="ixi")
            iyi = rpool.tile([P, HEADS, 8], i32, tag="iyi")
            nc.vector.tensor_copy(ixi[:], ix[:])
            nc.vector.tensor_copy(iyi[:], iy[:])

            ohx = eqpool.tile([P, HEADS, 8, 8], i32, tag="ohx")
            ohy = eqpool.tile([P, HEADS, 8, 8], i32, tag="ohy")
            nc.vector.tensor_tensor(ohx[:], pkh[:, :, :, None].to_broadcast([P, HEADS, 8, 8]),
                                    iota8[:, None, None, :].to_broadcast([P, HEADS, 8, 8]),
                                    OP.is_equal)
            nc.vector.tensor_tensor(ohy[:], pkl[:, :, :, None].to_broadcast([P, HEADS, 8, 8]),
                                    iota8[:, None, None, :].to_broadcast([P, HEADS, 8, 8]),
                                    OP.is_equal)
            mx = eqpool.tile([P, HEADS, 8, 8], i32, tag="mx")
            my = eqpool.tile([P, HEADS, 8, 8], i32, tag="my")
            nc.vector.tensor_tensor(mx[:], ohx[:],
                                    ixi[:, :, None, :].to_broadcast([P, HEADS, 8, 8]),
                                    OP.mult)
            nc.vector.tensor_tensor(my[:], ohy[:],
                                    iyi[:, :, None, :].to_broadcast([P, HEADS, 8, 8]),
                                    OP.mult)
            ixs = rpool.tile([P, HEADS, 8], i32, tag="ixs")
            iys = rpool.tile([P, HEADS, 8], i32, tag="iys")
            e8i = rpool.tile([P, HEADS, 8], i32, tag="e8i")
            with nc.allow_low_precision(reason="int32 onehot-select, exact"):
                nc.vector.tensor_reduce(ixs[:], mx[:], axis=AX.X, op=OP.add)
                nc.vector.tensor_reduce(iys[:], my[:], axis=AX.X, op=OP.add)
            nc.vector.scalar_tensor_tensor(e8i[:], ixs[:], NK, iys[:],
                                           op0=OP.mult, op1=OP.add)

            # merge heads: summed probs at duplicate experts, late dups killed
            e32 = e8i[:].rearrange("p h k -> p (h k)")
            p32 = p8v[:].rearrange("p h k -> p (h k)")
            eqf = eqpool.tile([P, 32, 32], f32, tag="eqf")
            nc.vector.tensor_tensor(eqf[:], e32[:, :, None].to_broadcast([P, 32, 32]),
                                    e32[:, None, :].to_broadcast([P, 32, 32]),
                                    OP.is_equal)
            pmm = eqpool.tile([P, 32, 32], f32, tag="pmm")
            nc.vector.tensor_tensor(pmm[:], eqf[:],
                                    p32[:, None, :].to_broadcast([P, 32, 32]), OP.mult)
            ptot = rpool.tile([P, 32], f32, tag="ptot")
            nc.vector.tensor_reduce(ptot[:], pmm[:], axis=AX.X, op=OP.add)
            nc.vector.tensor_copy(pv_all[:, c, :], ptot[:])

            eql = eqpool.tile([P, 32, 32], f32, tag="pmm")
            nc.vector.tensor_tensor(eql[:], eqf[:], ltf[:], OP.mult)
            dup = rpool.tile([P, 32], f32, tag="dup")
            nc.vector.tensor_reduce(dup[:], eql[:], axis=AX.X, op=OP.max)
            dupi = rpool.tile([P, 32], i32, tag="dupi")
            nc.vector.tensor_copy(dupi[:], dup[:])
            dupneg = rpool.tile([P, 32], i32, tag="dupneg")
            nc.vector.tensor_scalar(dupneg[:], dupi[:], -4096, None, op0=OP.mult)

            # quarter-local scatter indices, wrong-quarter/late-dup -> negative
            t1 = rpool.tile([P, 4, 32], i32, tag="t1")
            nc.vector.tensor_tensor(t1[:], e32[:, None, :].to_broadcast([P, 4, 32]),
                                    qoff[:, :, None].to_broadcast([P, 4, 32]),
                                    OP.subtract)
            t2 = rpool.tile([P, 4, 32], i32, tag="t2")
            nc.vector.tensor_scalar(t2[:], t1[:], 4095, None, op0=OP.bitwise_and)
            t3 = rpool.tile([P, 4, 32], i32, tag="t3")
            nc.vector.tensor_scalar(t3[:], t2[:], 3072, None, op0=OP.subtract)
            with nc.allow_low_precision(reason="int32 index math, exact"):
                nc.vector.tensor_tensor(idx_all[:, c, :, :], t3[:],
                                        dupneg[:, None, :].to_broadcast([P, 4, 32]),
                                        OP.add)

        def _emit_body():
            # ---------- phase A: sim matmuls (PE) + immediate DVE copies ----
            for c in range(TCH):
                ps = psum.tile([P, 512], f32, tag="ps")
                for kk in range(HK):
                    nc.tensor.matmul(ps[:], hsTb_sb[:, kk, c * P:(c + 1) * P],
                                     wk_sb[:, kk, :],
                                     start=(kk == 0), stop=(kk == HK - 1))
                nc.vector.tensor_copy(sim_all[:, c, :], ps[:])
            # ---------- phase B: all-expert logits (fp8 DoubleRow) ----------
            # single pass over expert columns with all 4 token chunks inside:
            # de is streamed once (4MB); the 4 lg buffers hold every chunk's
            # logits so routing + w-chains run strictly after B on DVE/ACT/
            # GPSIMD while the PE moves on to phase C.
            lg_t = [lgpool.tile([P, E], bf16, tag="lg", name=f"lg{c}")
                    for c in range(TCH)]
            for ec in range(E // 512):
                de_t = streams.tile([P, HK // 2, 2, 512], fp8, tag="de_t")
                nc.sync.dma_start(de_t[:], deq_r[:, :, :, ec * 512:(ec + 1) * 512])
                for c in range(TCH):
                    ps = psum.tile([P, 512], f32, tag="ps")
                    for j in range(HK // 2):
                        nc.tensor.matmul(ps[:], hsq_sb[:, j, :, c * P:(c + 1) * P],
                                         de_t[:, j, :, :], start=(j == 0),
                                         stop=(j == HK // 2 - 1), perf_mode=DR)
                    nc.vector.tensor_copy(lg_t[c][:, ec * 512:(ec + 1) * 512], ps[:])

            # routing (DVE) + w-chains (GPSIMD scatter, DVE mult/cast, ACT silu,
            # xbar transpose) — all off the PE's critical path
            for c in range(TCH):
                _routing(c)
            for c in range(TCH):
                pmap = pmpool.tile([P, E], bf16, tag="pmap")
                for qq in range(4):
                    nc.gpsimd.local_scatter(
                        pmap[:, qq * QE:(qq + 1) * QE], pv_all[:, c, :],
                        idx_all[:, c, qq, :], channels=P, num_elems=QE,
                        num_idxs=32)
                tt = scpool.tile([P, E], bf16, tag="tt")
                nc.vector.tensor_tensor(tt[:], lg_t[c][:], pmap[:], OP.mult)
                nc.scalar.activation(tt[:], tt[:], AF.Silu)
                sTb = scpool.tile([P, ICH, P], bf16, tag="sTb")
                # quarter-granularity transposes so stream DMAs interleave
                # on the shared DMA engines instead of stalling ~3.6us
                for tq in range(4):
                    nc.scalar.dma_start_transpose(
                        sTb[:, tq * (ICH // 4):(tq + 1) * (ICH // 4), :],
                        tt[:, tq * (E // 4):(tq + 1) * (E // 4)])
                nc.vector.tensor_scalar(sTq[:, :, c * P:(c + 1) * P], sTb[:],
                                        S_SC, None, op0=OP.mult)

            # ---------- phase C: dense up-proj + silu ----------
            for ic8 in range(ICH // 4):
                wup_t = wupstr.tile([P, HK, 4 * P], bf16, tag="wup_t")
                nc.sync.dma_start(wup_t[:], wupT_r[:, :, ic8 * 4 * P:(ic8 + 1) * 4 * P])
                for j in range(4):
                    ic = ic8 * 4 + j
                    ps = psum.tile([P, 512], f32, tag="ps")
                    for kk in range(HK):
                        nc.tensor.matmul(ps[:], wup_t[:, kk, j * P:(j + 1) * P],
                                         hsTb_sb[:, kk, :],
                                         start=(kk == 0), stop=(kk == HK - 1))
                    nc.scalar.activation(y1T[:, ic, :], ps[:], AF.Silu)

            # ---------- phase D: down-proj + expert combine, fused in PSUM ----------
            # All 8 output d-chunks accumulate concurrently (8 PSUM banks) so each
            # weight row-block is loaded once, in one large DMA.
            ps_d = [psum.tile([P, 512], f32, tag="ps", name=f"ps_d{dc}")
                    for dc in range(DCH)]
            for ic2 in range(ICH // 2):
                wd_t = wstream.tile([P, 2, H], bf16, tag="wd_t")
                nc.sync.dma_start(wd_t[:], wdownT_r[:, 2 * ic2:2 * ic2 + 2, :])
                for i2 in range(2):
                    ic = 2 * ic2 + i2
                    for dc in range(DCH):
                        nc.tensor.matmul(ps_d[dc][:], wd_t[:, i2, dc * P:(dc + 1) * P],
                                         y1T[:, ic, :], start=(ic == 0), stop=False)
            for j2 in range(E // 512):
                ue_t = wstream.tile([P, 2, 2, H], fp8, tag="ue_t")
                nc.sync.dma_start(ue_t[:], ueq_r[:, 2 * j2:2 * j2 + 2, :, :])
                for i2 in range(2):
                    j = 2 * j2 + i2
                    for dc in range(DCH):
                        nc.tensor.matmul(ps_d[dc][:], ue_t[:, i2, :, dc * P:(dc + 1) * P],
                                         sTq[:, 2 * j:2 * j + 2, :], start=False,
                                         stop=(j == E // 256 - 1), perf_mode=DR)
            for dc in range(DCH):
                ot = outp.tile([P, 512], f32, tag="ot")
                nc.scalar.activation(ot[:], ps_d[dc][:], AF.Copy, scale=OUT_DESC)
                nc.gpsimd.dma_start(outT_d[dc * P:(dc + 1) * P, :], ot[:])

        if loop_n is not None:
            with tc.For_i(0, loop_n, 1):
                _emit_body()
        else:
            for _rep in range(repeat):
                _emit_body()

    nc.compile()
    return nc


def _host_prep(hidden_states, W_up, W_down, W_q, keys, down_embed, up_embed):
    bf = ml_dtypes.bfloat16
    f8 = ml_dtypes.float8_e4m3
    hs = np.asarray(hidden_states, dtype=np.float32).reshape(B * T, H)
    W_up = np.asarray(W_up, dtype=np.float32)
    W_down = np.asarray(W_down, dtype=np.float32)
    W_q = np.asarray(W_q, dtype=np.float32)
    keys = np.asarray(keys, dtype=np.float32)
    down_embed = np.asarray(down_embed, dtype=np.float32)
    up_embed = np.asarray(up_embed, dtype=np.float32)

    # compose product-key similarity: WK[(p2,h,k), d] = sum_r Wq[(p2,h,r), d]*keys[h,k,p2,r]
    Wq3 = W_q.reshape(2, HEADS, NK, H).astype(np.float64)
    WK = np.einsum("phrd,hkpr->phkd", Wq3, keys.astype(np.float64))
    WK_T = np.ascontiguousarray(WK.reshape(512, H).T).astype(np.float32)  # [H, 512]

    shared = {
        "wk": WK_T.astype(bf),
        "wupT": np.ascontiguousarray(W_up.T).astype(bf),                   # [H, I]
        "wdownT": np.ascontiguousarray(W_down.T * (S_SC * UE_SC)).astype(bf),
        "deq": np.ascontiguousarray(down_embed.T * DE_SC).astype(f8),      # [H, E]
        "ueq": np.ascontiguousarray(up_embed * UE_SC).astype(f8),          # [E, H]
    }
    in_maps = []
    for i in range(NCORES):
        shard = hs[i * NT:(i + 1) * NT]                              # [NT, H]
        hsT = np.ascontiguousarray(shard.T)                          # [H, NT]
        m = dict(shared)
        m["hsT_b"] = hsT.astype(bf)
        m["hsT_q"] = (hsT * HS_SC).astype(f8)
        in_maps.append(m)
    return in_maps


def kernel(hidden_states, W_up, W_down, W_q, keys, down_embed, up_embed,
           trace=False):
    from concourse.bass_utils import run_bass_kernel_spmd

    if "nc" not in _CACHE:
        _CACHE["nc"] = _build_program()
    nc = _CACHE["nc"]

    in_maps = _host_prep(hidden_states, W_up, W_down, W_q, keys,
                         down_embed, up_embed)
    res = run_bass_kernel_spmd(nc, in_maps, list(range(NCORES)), trace=trace)
    out = np.empty((B * T, H), np.float32)
    for i, r in enumerate(res.results):
        out[i * NT:(i + 1) * NT] = r["outT"].T
    if trace:
        kernel.last_results = res
    return out.reshape(B, T, H)
